# revision 6
# baseline (speedup 1.0000x reference)
"""Trainium2 Bass kernel for nn_AUTOGCNLayer (GCN layer with two message-passing
rounds, three weight branches and mutual sigmoid gating).

Strategy (8 NeuronCores, node-parallel):
  - Nodes are sharded contiguously: core c owns rows [c*6250, (c+1)*6250),
    padded to 6272 = 49*128 rows per core.
  - Host-side preprocessing (index bookkeeping only): per core, edges are
    bucketed by destination window (128 nodes), split into lo/hi halves by
    source row (int16 gather index limit), padded to 128-edge chunks with a
    schedule that is identical across cores (SPMD single NEFF).
  - deg (edge counts per node, a counting-sort byproduct) ships as int32;
    all float math (clip, rsqrt, feature scaling, ...) happens on device.
  - On device, per round: dma_gather pulls per-edge source rows from the
    AllGather'ed [8*6272, 128] normalized-feature table; a one-hot matrix
    built on DVE (iota == dstoff) is the stationary matmul operand that
    scatter-accumulates each 128-edge chunk into its PSUM window
    (segment-sum on the tensor engine).
  - Epilogue per window: u = a*h + b*x combos, PE transpose, 128x128 weight
    matmuls in transposed layout, mutual sigmoid gating, bias, snorm, relu.
"""

import numpy as np

N = 50000
E = 600000
D = 128
KG = 8
EPS = 1e-09
NCORES = 8
NPC = N // NCORES            # 6250 real nodes per core
WINDOWS = (NPC + 127) // 128  # 49
NPAD = WINDOWS * 128          # 6272 padded nodes per core
SPLIT = 32768                 # lo/hi source split (int16 gather index limit)
AGROWS = NCORES * NPAD        # 50176 rows in the AllGather'ed feature table
MAX_CALL_CHUNKS = 8           # <=1024 indices per dma_gather call
NQUEUES = 4


# ---------------------------------------------------------------------------
# Host-side graph preprocessing (pure index bookkeeping)
# ---------------------------------------------------------------------------

def _preprocess(src, dst):
    """Build the shared chunk schedule and per-core index tensors.

    Returns (schedule, per_core) where schedule is shared across cores:
      chunk_wh: list of (window, half) per chunk slot
      calls:    list of (chunk_start, n_chunks, half) per dma_gather call
    and per_core[c] has:
      deg:    [128, WINDOWS] int32 in-degree (padded nodes 0)
      dstoff: [128, C_total] float32 (pad slots -1.0)
      gidx:   [128, C_total*8] int16 gather indices in dma_gather layout
    """
    src = np.asarray(src, dtype=np.int64)
    dst = np.asarray(dst, dtype=np.int64)
    core = dst // NPC
    ldst = dst - core * NPC
    win = ldst // 128
    woff = ldst - win * 128
    # padded global source row (AllGather table indexing)
    psrc = (src // NPC) * NPAD + (src % NPC)
    half = (psrc >= SPLIT).astype(np.int64)

    counts = np.zeros((NCORES, WINDOWS, 2), dtype=np.int64)
    np.add.at(counts, (core, win, half), 1)
    kchunks = (counts + 127) // 128          # chunks per (core, window, half)
    Kwh = kchunks.max(axis=0)                 # shared schedule: [WINDOWS, 2]

    chunk_wh = []
    for w in range(WINDOWS):
        for h in (0, 1):
            chunk_wh.extend([(w, h)] * int(Kwh[w, h]))
    C_total = len(chunk_wh)

    # call list: contiguous same-half chunk runs, <= MAX_CALL_CHUNKS each
    calls = []
    c0 = 0
    while c0 < C_total:
        h = chunk_wh[c0][1]
        c1 = c0
        while c1 < C_total and chunk_wh[c1][1] == h and c1 - c0 < MAX_CALL_CHUNKS:
            c1 += 1
        calls.append((c0, c1 - c0, h))
        c0 = c1

    # block start offsets in the padded edge array, shared schedule
    block_start = np.zeros((WINDOWS, 2), dtype=np.int64)
    acc = 0
    for w in range(WINDOWS):
        for h in (0, 1):
            block_start[w, h] = acc
            acc += int(Kwh[w, h]) * 128
    total_slots = acc
    assert acc == C_total * 128

    # in-degree per local node (counting-sort byproduct)
    deg_all = np.zeros(NCORES * NPC, dtype=np.int64)
    np.add.at(deg_all, dst, 1)

    per_core = []
    for c in range(NCORES):
        m = core == c
        w_c, h_c, off_c, psrc_c = win[m], half[m], woff[m], psrc[m]
        # stable ordering by (window, half) via counting offsets
        order = np.lexsort((psrc_c, h_c, w_c))
        w_c, h_c, off_c, psrc_c = w_c[order], h_c[order], off_c[order], psrc_c[order]
        # slot position: block_start[w,h] + rank within block
        key = w_c * 2 + h_c
        # rank within each (w,h) group (edges already sorted by key)
        starts = np.searchsorted(key, np.arange(WINDOWS * 2))
        rank = np.arange(len(key)) - starts[key]
        slot = block_start[w_c, h_c] + rank

        dstoff_full = np.full(total_slots, -1.0, dtype=np.float32)
        gsrc_full = np.zeros(total_slots, dtype=np.int64)
        dstoff_full[slot] = off_c.astype(np.float32)
        gsrc_full[slot] = psrc_c - h_c * SPLIT  # rebase hi half
        assert gsrc_full.max(initial=0) < SPLIT

        # dstoff layout [128, C_total]: slot s = chunk s//128, partition s%128
        dstoff_arr = dstoff_full.reshape(C_total, 128).T.copy()

        # gather idx layout per call: linear idx i (over the call's slots)
        # lives at partition i%16 (replicated x8), free col call_base + i//16
        gidx_arr = np.zeros((128, C_total * 8), dtype=np.int16)
        for (cs, nch, _h) in calls:
            lin = gsrc_full[cs * 128:(cs + nch) * 128].astype(np.int16)
            wrapped = lin.reshape(nch * 8, 16).T  # [16, nch*8]
            gidx_arr[:, cs * 8:(cs + nch) * 8] = np.tile(wrapped, (8, 1))

        deg_c = np.zeros(NPAD, dtype=np.int32)
        deg_c[:NPC] = deg_all[c * NPC:(c + 1) * NPC]
        deg_arr = deg_c.reshape(WINDOWS, 128).T.copy()  # [128, WINDOWS]

        per_core.append({"deg": deg_arr, "dstoff": dstoff_arr, "gidx": gidx_arr})

    return {"chunk_wh": chunk_wh, "calls": calls, "C_total": C_total}, per_core


# ---------------------------------------------------------------------------
# Bass kernel builder
# ---------------------------------------------------------------------------

def _build(schedule, repeat_body=1, skip_collectives=False, bench_mode=None):
    import concourse.bacc as bacc
    import concourse.mybir as mybir
    import concourse.tile as tile
    from concourse.masks import make_identity

    chunk_wh = schedule["chunk_wh"]
    calls = schedule["calls"]
    C_total = schedule["C_total"]
    f32 = mybir.dt.float32
    bf16 = mybir.dt.bfloat16
    AF = mybir.ActivationFunctionType
    OP = mybir.AluOpType

    nc = bacc.Bacc("TRN2", debug=False, num_swdge_queues=NQUEUES)

    feat_in = nc.dram_tensor("feature", [NPAD, D], f32, kind="ExternalInput")
    snorm_in = nc.dram_tensor("snorm", [128, WINDOWS], f32, kind="ExternalInput")
    deg_in = nc.dram_tensor("deg", [128, WINDOWS], mybir.dt.int32, kind="ExternalInput")
    dstoff_in = nc.dram_tensor("dstoff", [128, C_total], f32, kind="ExternalInput")
    gidx_in = nc.dram_tensor("gidx", [128, C_total * 8], mybir.dt.int16, kind="ExternalInput")
    wlT_in = nc.dram_tensor("W_lowT", [D, D], f32, kind="ExternalInput")
    whT_in = nc.dram_tensor("W_highT", [D, D], f32, kind="ExternalInput")
    wmT_in = nc.dram_tensor("W_midT", [D, D], f32, kind="ExternalInput")
    gl_in = nc.dram_tensor("gamma_low", [1, KG], f32, kind="ExternalInput")
    gh_in = nc.dram_tensor("gamma_high", [1, KG], f32, kind="ExternalInput")
    gm_in = nc.dram_tensor("gamma_mid", [1, KG], f32, kind="ExternalInput")
    bias_in = nc.dram_tensor("bias", [128, 1], f32, kind="ExternalInput")
    out_dram = nc.dram_tensor("out", [NPAD, D], f32, kind="ExternalOutput")

    alpha = np.linspace(EPS, 1.0 - EPS, KG)
    midalpha = np.linspace(EPS, 1.0, KG)

    with tile.TileContext(nc) as tc:
        with (
            tc.tile_pool(name="const", bufs=1) as constp,
            tc.tile_pool(name="big", bufs=1) as bigp,
            tc.tile_pool(name="msg", bufs=12) as msgp,
            tc.tile_pool(name="oh", bufs=8) as ohp,
            tc.tile_pool(name="wrk", bufs=4) as wrkp,
            tc.tile_pool(name="pswin", bufs=2, space="PSUM") as pswin,
            tc.tile_pool(name="psep", bufs=2, space="PSUM") as psep,
            tc.tile_pool(name="dram", bufs=1, space="DRAM") as dramp,
        ):
            # ---------------- constants / small tiles ----------------
            iota_i = constp.tile([128, 128], mybir.dt.int32)
            nc.gpsimd.iota(iota_i[:], pattern=[[1, 128]], base=0, channel_multiplier=0)
            iota_f = constp.tile([128, 128], f32)
            nc.vector.tensor_copy(iota_f[:], iota_i[:])
            ident = constp.tile([128, 128], f32)
            make_identity(nc, ident[:])

            dstoff_t = constp.tile([128, C_total], f32)
            nc.sync.dma_start(dstoff_t[:], dstoff_in[:])
            gidx_t = constp.tile([128, C_total * 8], mybir.dt.int16)
            nc.sync.dma_start(gidx_t[:], gidx_in[:])
            snorm_t = constp.tile([128, WINDOWS], f32)
            nc.sync.dma_start(snorm_t[:], snorm_in[:])
            bias_t = constp.tile([128, 1], f32)
            nc.sync.dma_start(bias_t[:], bias_in[:])
            wT = {}
            for nm, drt in (("low", wlT_in), ("high", whT_in), ("mid", wmT_in)):
                t = constp.tile([128, 128], f32, tag=f"w{nm}")
                nc.sync.dma_start(t[:], drt[:])
                wT[nm] = t

            # deg -> norm, norm^2   [128, WINDOWS]
            deg_t = constp.tile([128, WINDOWS], mybir.dt.int32)
            nc.sync.dma_start(deg_t[:], deg_in[:])
            deg_f = constp.tile([128, WINDOWS], f32)
            nc.vector.tensor_copy(deg_f[:], deg_t[:])
            nc.vector.tensor_scalar_max(deg_f[:], deg_f[:], 1.0)
            sq = constp.tile([128, WINDOWS], f32)
            nc.scalar.activation(sq[:], deg_f[:], AF.Sqrt)
            norm_t = constp.tile([128, WINDOWS], f32)
            nc.vector.reciprocal(norm_t[:], sq[:])
            norm2_t = constp.tile([128, WINDOWS], f32)
            nc.vector.tensor_tensor(norm2_t[:], norm_t[:], norm_t[:], OP.mult)

            # ---------------- epilogue scalar coefficients ----------------
            # broadcast gammas to [128, KG] via K=1 matmul with ones lhsT
            ones_row = constp.tile([1, 128], f32)
            nc.vector.memset(ones_row[:], 1.0)
            coeff = {}
            for nm, drt in (("low", gl_in), ("high", gh_in), ("mid", gm_in)):
                g_small = constp.tile([1, KG], f32, tag=f"gs{nm}")
                nc.sync.dma_start(g_small[:], drt[:])
                g_ps = psep.tile([128, KG], f32, space="PSUM", tag="bT")
                nc.tensor.matmul(g_ps[:], lhsT=ones_row[:], rhs=g_small[:],
                                 start=True, stop=True)
                g_b = constp.tile([128, KG], f32, tag=f"gb{nm}")
                nc.scalar.activation(g_b[:], g_ps[:], AF.Relu)
                coeff[nm] = g_b

            def dotcol(gb, weights, tag):
                wt = constp.tile([128, KG], f32, tag=f"wt{tag}")
                for i, v in enumerate(weights):
                    nc.vector.memset(wt[:, i:i + 1], float(v))
                prod = constp.tile([128, KG], f32, tag=f"pr{tag}")
                nc.vector.tensor_tensor(prod[:], gb[:], wt[:], OP.mult)
                col = constp.tile([128, 1], f32, tag=f"col{tag}")
                nc.vector.tensor_reduce(col[:], prod[:], mybir.AxisListType.X, OP.add)
                return col

            a0_col = dotcol(coeff["low"], alpha, "a0")
            b0_col = dotcol(coeff["low"], 1.0 - alpha, "b0")
            a1p_col = dotcol(coeff["high"], alpha, "a1p")   # positive; negated below
            b1_col = dotcol(coeff["high"], 1.0 - alpha, "b1")
            c2_col = dotcol(coeff["mid"], np.ones(KG), "c2")
            d2_col = dotcol(coeff["mid"], midalpha, "d2")
            a1_col = constp.tile([128, 1], f32)
            nc.vector.tensor_scalar_mul(a1_col[:], a1p_col[:], -1.0)

            # ---------------- load feature tiles ----------------
            x_buf = bigp.tile([128, NPAD], f32)   # window w at [:, w*128:(w+1)*128]
            nc.sync.dma_start(
                x_buf[:].rearrange("p (w d) -> p w d", d=D),
                feat_in[:].rearrange("(w p) d -> p w d", p=128),
            )

            # normfeat slice -> DRAM bounce, AllGather -> ag1
            bounce1 = dramp.tile([NPAD, D], bf16)
            for w in range(WINDOWS):
                nf = wrkp.tile([128, 128], bf16, tag="nf")
                nc.scalar.activation(nf[:], x_buf[:, w * 128:(w + 1) * 128], AF.Copy,
                                     scale=norm_t[:, w:w + 1])
                nc.sync.dma_start(bounce1[w * 128:(w + 1) * 128, :], nf[:])
            ag1 = dramp.tile([AGROWS, D], bf16)
            if skip_collectives:
                nc.gpsimd.dma_start(ag1[0:NPAD, :], bounce1[:, :])
            else:
                nc.gpsimd.collective_compute(
                    "AllGather", mybir.AluOpType.bypass,
                    ins=[bounce1.opt()], outs=[ag1.opt()],
                    replica_groups=[list(range(NCORES))],
                )

            h_buf = bigp.tile([128, NPAD], f32)
            h1_buf = bigp.tile([128, NPAD], f32)
            bounce2 = dramp.tile([NPAD, D], bf16)
            ag2 = dramp.tile([AGROWS, D], bf16)

            def mp_round(ag_src, out_h_buf, write_normh):
                """One message-passing round from gather table ag_src."""
                # issue all gather calls; msg tiles keyed by call index
                msg_tiles = {}
                dummy_msg = None
                if bench_mode == "scatter":
                    dummy_msg = msgp.tile([128, MAX_CALL_CHUNKS, 128], bf16, tag="msg")
                    nc.vector.memset(dummy_msg[:].rearrange("p c d -> p (c d)"), 0.5)
                for ci, (cs, nch, h) in enumerate(calls):
                    if bench_mode == "scatter":
                        for k in range(nch):
                            msg_tiles[cs + k] = (dummy_msg, k)
                        continue
                    mt = msgp.tile([128, MAX_CALL_CHUNKS, 128], bf16, tag="msg")
                    base = ag_src[SPLIT:, :] if h else ag_src[:SPLIT, :]
                    num_idxs = nch * 128
                    nc.gpsimd.dma_gather(
                        mt[:, :nch, :], base, gidx_t[:, cs * 8:cs * 8 + nch * 8],
                        num_idxs, num_idxs, D, queue_num=ci % NQUEUES,
                    )
                    if bench_mode == "gather":
                        red = ohp.tile([128, 1], f32, tag="gred")
                        nc.vector.tensor_reduce(
                            red[:], mt[:, 0, ::32], mybir.AxisListType.X,
                            OP.add)
                        nc.vector.tensor_tensor(
                            out_h_buf[:, 0:1], out_h_buf[:, 0:1], red[:], OP.add)
                        continue
                    for k in range(nch):
                        msg_tiles[cs + k] = (mt, k)
                if bench_mode == "gather":
                    return

                # chunk->window accumulation
                cur_w = -1
                psum_w = None
                wstart = {}
                for c0, (w, h) in enumerate(chunk_wh):
                    if w != cur_w:
                        cur_w = w
                        psum_w = pswin.tile([128, 128], f32, space="PSUM", tag="agg")
                        wstart[w] = True
                    mt, k = msg_tiles[c0]
                    oh = ohp.tile([128, 128], bf16, tag="oh")
                    nc.vector.tensor_scalar(oh[:], iota_f[:], dstoff_t[:, c0:c0 + 1],
                                            None, OP.is_equal)
                    last = (c0 + 1 == len(chunk_wh)) or chunk_wh[c0 + 1][0] != w
                    nc.tensor.matmul(psum_w[:], lhsT=oh[:], rhs=mt[:, k, :],
                                     start=wstart.pop(w, False), stop=last)
                    if last:
                        nc.scalar.activation(
                            out_h_buf[:, w * 128:(w + 1) * 128], psum_w[:],
                            AF.Copy, scale=norm_t[:, w:w + 1])
                        if write_normh:
                            nh = wrkp.tile([128, 128], bf16, tag="nh")
                            nc.scalar.activation(nh[:], psum_w[:], AF.Copy,
                                                 scale=norm2_t[:, w:w + 1])
                            nc.sync.dma_start(
                                bounce2[w * 128:(w + 1) * 128, :], nh[:])

            for _rep in range(repeat_body):
                if bench_mode == "epilogue":
                    nc.vector.memset(h_buf[:, 0:1], 0.1)
                    nc.vector.memset(h1_buf[:, 0:1], 0.1)
                elif bench_mode == "ag":
                    nc.gpsimd.dma_start(bounce2[0:128, :], x_buf[:, 0:128])
                    nc.gpsimd.collective_compute(
                        "AllGather", mybir.AluOpType.bypass,
                        ins=[bounce2.opt()], outs=[ag2.opt()],
                        replica_groups=[list(range(NCORES))],
                    )
                    continue
                else:
                    mp_round(ag1, h_buf, write_normh=True)
                    if skip_collectives:
                        nc.gpsimd.dma_start(ag2[0:NPAD, :], bounce2[:, :])
                    else:
                        nc.gpsimd.collective_compute(
                            "AllGather", mybir.AluOpType.bypass,
                            ins=[bounce2.opt()], outs=[ag2.opt()],
                            replica_groups=[list(range(NCORES))],
                        )
                    mp_round(ag2, h1_buf, write_normh=False)
                if bench_mode in ("gather", "scatter", "rounds"):
                    continue

                # ---------------- epilogue per window ----------------
                for w in range(WINDOWS):
                    sl = slice(w * 128, (w + 1) * 128)
                    x_w = x_buf[:, sl]
                    h_w = h_buf[:, sl]
                    h1_w = h1_buf[:, sl]

                    def combo(in_hi, a_col, x_col, op1, tag):
                        xb = wrkp.tile([128, 128], f32, tag=f"xb{tag}")
                        nc.vector.tensor_scalar_mul(xb[:], x_w, x_col[:])
                        u = wrkp.tile([128, 128], f32, tag=f"u{tag}")
                        nc.vector.scalar_tensor_tensor(
                            out=u[:], in0=in_hi, scalar=a_col[:], in1=xb[:],
                            op0=OP.mult, op1=op1)
                        return u

                    u0 = combo(h_w, a0_col, b0_col, OP.add, "0")
                    u1 = combo(h_w, a1_col, b1_col, OP.add, "1")
                    u2 = combo(h1_w, c2_col, d2_col, OP.subtract, "2")

                    oT = {}
                    for nm, u in (("low", u0), ("high", u1), ("mid", u2)):
                        up = psep.tile([128, 128], f32, space="PSUM", tag="uT")
                        nc.tensor.transpose(up[:], u[:], ident[:])
                        uT = wrkp.tile([128, 128], f32, tag=f"uT{nm}")
                        nc.vector.tensor_copy(uT[:], up[:])
                        op = psep.tile([128, 128], f32, space="PSUM", tag="om")
                        nc.tensor.matmul(op[:], lhsT=wT[nm][:], rhs=uT[:],
                                         start=True, stop=True)
                        ot = wrkp.tile([128, 128], f32, tag=f"ot{nm}")
                        nc.scalar.copy(ot[:], op[:])
                        oT[nm] = ot

                    # mutual gating (T layout)
                    tmp = wrkp.tile([128, 128], f32, tag="gt")
                    sig = wrkp.tile([128, 128], f32, tag="gs")
                    nc.vector.tensor_tensor(tmp[:], oT["high"][:], oT["mid"][:], OP.add)
                    nc.scalar.activation(sig[:], tmp[:], AF.Sigmoid)
                    nc.vector.tensor_tensor(oT["low"][:], oT["low"][:], sig[:], OP.mult)
                    nc.vector.tensor_tensor(tmp[:], oT["low"][:], oT["mid"][:], OP.add)
                    nc.scalar.activation(sig[:], tmp[:], AF.Sigmoid)
                    nc.vector.tensor_tensor(oT["high"][:], oT["high"][:], sig[:], OP.mult)
                    nc.vector.tensor_tensor(tmp[:], oT["low"][:], oT["high"][:], OP.add)
                    nc.scalar.activation(sig[:], tmp[:], AF.Sigmoid)
                    nc.vector.tensor_tensor(oT["mid"][:], oT["mid"][:], sig[:], OP.mult)

                    nc.vector.tensor_tensor(tmp[:], oT["low"][:], oT["high"][:], OP.add)
                    nc.vector.tensor_tensor(tmp[:], tmp[:], oT["mid"][:], OP.add)
                    nc.vector.tensor_scalar_add(tmp[:], tmp[:], bias_t[:])

                    # back to row layout; relu(x * snorm)
                    bp = psep.tile([128, 128], f32, space="PSUM", tag="bT")
                    nc.tensor.transpose(bp[:], tmp[:], ident[:])
                    outt = wrkp.tile([128, 128], f32, tag="outt")
                    nc.scalar.activation(outt[:], bp[:], AF.Relu,
                                         scale=snorm_t[:, w:w + 1])
                    nc.sync.dma_start(out_dram[w * 128:(w + 1) * 128, :], outt[:])

    nc.compile()
    return nc


# ---------------------------------------------------------------------------
# Public entry point
# ---------------------------------------------------------------------------

def kernel(feature, snorm_n, src, dst, W_low, W_high, W_mid,
           gamma_low, gamma_high, gamma_mid, bias):
    from concourse.bass_utils import run_bass_kernel_spmd

    feature = np.asarray(feature, dtype=np.float32)
    snorm_n = np.asarray(snorm_n, dtype=np.float32)
    schedule, per_core = _preprocess(np.asarray(src), np.asarray(dst))
    nc = _build(schedule)

    in_maps = []
    for c in range(NCORES):
        feat_c = np.zeros((NPAD, D), np.float32)
        feat_c[:NPC] = feature[c * NPC:(c + 1) * NPC]
        sn_c = np.zeros(NPAD, np.float32)
        sn_c[:NPC] = snorm_n[c * NPC:(c + 1) * NPC, 0]
        in_maps.append({
            "feature": feat_c,
            "snorm": sn_c.reshape(WINDOWS, 128).T.copy(),
            "deg": per_core[c]["deg"],
            "dstoff": per_core[c]["dstoff"],
            "gidx": per_core[c]["gidx"],
            "W_lowT": np.ascontiguousarray(np.asarray(W_low, np.float32).T),
            "W_highT": np.ascontiguousarray(np.asarray(W_high, np.float32).T),
            "W_midT": np.ascontiguousarray(np.asarray(W_mid, np.float32).T),
            "gamma_low": np.asarray(gamma_low, np.float32).reshape(1, KG),
            "gamma_high": np.asarray(gamma_high, np.float32).reshape(1, KG),
            "gamma_mid": np.asarray(gamma_mid, np.float32).reshape(1, KG),
            "bias": np.asarray(bias, np.float32).reshape(128, 1),
        })

    res = run_bass_kernel_spmd(nc, in_maps, core_ids=list(range(NCORES)))
    out = np.concatenate(
        [res.results[c]["out"][:NPC] for c in range(NCORES)], axis=0)
    return out


# revision 8
# speedup vs baseline: 1.2585x; 1.2585x over previous
"""Trainium2 Bass kernel for nn_AUTOGCNLayer (GCN layer with two message-passing
rounds, three weight branches and mutual sigmoid gating).

Strategy (8 NeuronCores, node-parallel):
  - Nodes are sharded contiguously: core c owns rows [c*6250, (c+1)*6250),
    padded to 6272 = 49*128 rows per core.
  - Host-side preprocessing (index bookkeeping only): per core, edges are
    bucketed by destination window (128 nodes), split into lo/hi halves by
    source row (int16 gather index limit), padded to 128-edge chunks with a
    schedule that is identical across cores (SPMD single NEFF).
  - deg (edge counts per node, a counting-sort byproduct) ships as int32;
    all float math (clip, rsqrt, feature scaling, ...) happens on device.
  - On device, per round: dma_gather pulls per-edge source rows from the
    AllGather'ed [8*6272, 128] normalized-feature table; a one-hot matrix
    built on DVE (iota == dstoff) is the stationary matmul operand that
    scatter-accumulates each 128-edge chunk into its PSUM window
    (segment-sum on the tensor engine).
  - Epilogue per window: u = a*h + b*x combos, PE transpose, 128x128 weight
    matmuls in transposed layout, mutual sigmoid gating, bias, snorm, relu.
"""

import numpy as np

N = 50000
E = 600000
D = 128
KG = 8
EPS = 1e-09
NCORES = 8
NPC = N // NCORES            # 6250 real nodes per core
WINDOWS = (NPC + 127) // 128  # 49
NPAD = WINDOWS * 128          # 6272 padded nodes per core
SPLIT = 32768                 # lo/hi source split (int16 gather index limit)
AGROWS = NCORES * NPAD        # 50176 rows in the AllGather'ed feature table
MAX_CALL_CHUNKS = 8           # <=1024 indices per dma_gather call
NQUEUES = 4


# ---------------------------------------------------------------------------
# Host-side graph preprocessing (pure index bookkeeping)
# ---------------------------------------------------------------------------

def _preprocess(src, dst):
    """Build the shared chunk schedule and per-core index tensors.

    Returns (schedule, per_core) where schedule is shared across cores:
      chunk_wh: list of (window, half) per chunk slot
      calls:    list of (chunk_start, n_chunks, half) per dma_gather call
    and per_core[c] has:
      deg:    [128, WINDOWS] int32 in-degree (padded nodes 0)
      dstoff: [128, C_total] float32 (pad slots -1.0)
      gidx:   [128, C_total*8] int16 gather indices in dma_gather layout
    """
    src = np.asarray(src, dtype=np.int64)
    dst = np.asarray(dst, dtype=np.int64)
    core = dst // NPC
    ldst = dst - core * NPC
    win = ldst // 128
    woff = ldst - win * 128
    # padded global source row (AllGather table indexing)
    psrc = (src // NPC) * NPAD + (src % NPC)
    half = (psrc >= SPLIT).astype(np.int64)

    counts = np.zeros((NCORES, WINDOWS, 2), dtype=np.int64)
    np.add.at(counts, (core, win, half), 1)
    kchunks = (counts + 127) // 128          # chunks per (core, window, half)
    Kwh = kchunks.max(axis=0)                 # shared schedule: [WINDOWS, 2]

    chunk_wh = []
    for w in range(WINDOWS):
        for h in (0, 1):
            chunk_wh.extend([(w, h)] * int(Kwh[w, h]))
    C_total = len(chunk_wh)

    # call list: contiguous same-half chunk runs, <= MAX_CALL_CHUNKS each
    calls = []
    c0 = 0
    while c0 < C_total:
        h = chunk_wh[c0][1]
        c1 = c0
        while c1 < C_total and chunk_wh[c1][1] == h and c1 - c0 < MAX_CALL_CHUNKS:
            c1 += 1
        calls.append((c0, c1 - c0, h))
        c0 = c1

    # block start offsets in the padded edge array, shared schedule
    block_start = np.zeros((WINDOWS, 2), dtype=np.int64)
    acc = 0
    for w in range(WINDOWS):
        for h in (0, 1):
            block_start[w, h] = acc
            acc += int(Kwh[w, h]) * 128
    total_slots = acc
    assert acc == C_total * 128

    # in-degree per local node (counting-sort byproduct)
    deg_all = np.zeros(NCORES * NPC, dtype=np.int64)
    np.add.at(deg_all, dst, 1)

    per_core = []
    for c in range(NCORES):
        m = core == c
        w_c, h_c, off_c, psrc_c = win[m], half[m], woff[m], psrc[m]
        # stable ordering by (window, half) via counting offsets
        order = np.lexsort((psrc_c, h_c, w_c))
        w_c, h_c, off_c, psrc_c = w_c[order], h_c[order], off_c[order], psrc_c[order]
        # slot position: block_start[w,h] + rank within block
        key = w_c * 2 + h_c
        # rank within each (w,h) group (edges already sorted by key)
        starts = np.searchsorted(key, np.arange(WINDOWS * 2))
        rank = np.arange(len(key)) - starts[key]
        slot = block_start[w_c, h_c] + rank

        dstoff_full = np.full(total_slots, -1.0, dtype=np.float32)
        gsrc_full = np.zeros(total_slots, dtype=np.int64)
        dstoff_full[slot] = off_c.astype(np.float32)
        gsrc_full[slot] = psrc_c - h_c * SPLIT  # rebase hi half
        assert gsrc_full.max(initial=0) < SPLIT

        # dstoff layout [128, C_total]: slot s = chunk s//128, partition s%128
        dstoff_arr = dstoff_full.reshape(C_total, 128).T.copy()

        # gather idx layout per call: linear idx i (over the call's slots)
        # lives at partition i%16 (replicated x8), free col call_base + i//16
        gidx_arr = np.zeros((128, C_total * 8), dtype=np.int16)
        for (cs, nch, _h) in calls:
            lin = gsrc_full[cs * 128:(cs + nch) * 128].astype(np.int16)
            wrapped = lin.reshape(nch * 8, 16).T  # [16, nch*8]
            gidx_arr[:, cs * 8:(cs + nch) * 8] = np.tile(wrapped, (8, 1))

        deg_c = np.zeros(NPAD, dtype=np.int32)
        deg_c[:NPC] = deg_all[c * NPC:(c + 1) * NPC]
        deg_arr = deg_c.reshape(WINDOWS, 128).T.copy()  # [128, WINDOWS]

        per_core.append({"deg": deg_arr, "dstoff": dstoff_arr, "gidx": gidx_arr})

    return {"chunk_wh": chunk_wh, "calls": calls, "C_total": C_total}, per_core


# ---------------------------------------------------------------------------
# Bass kernel builder
# ---------------------------------------------------------------------------

def _build(schedule, repeat_body=1, skip_collectives=False, bench_mode=None):
    import concourse.bacc as bacc
    import concourse.mybir as mybir
    import concourse.tile as tile
    from concourse.masks import make_identity

    chunk_wh = schedule["chunk_wh"]
    calls = schedule["calls"]
    C_total = schedule["C_total"]
    f32 = mybir.dt.float32
    bf16 = mybir.dt.bfloat16
    AF = mybir.ActivationFunctionType
    OP = mybir.AluOpType

    nc = bacc.Bacc("TRN2", debug=False, num_swdge_queues=NQUEUES)

    feat_in = nc.dram_tensor("feature", [NPAD, D], f32, kind="ExternalInput")
    snorm_in = nc.dram_tensor("snorm", [128, WINDOWS], f32, kind="ExternalInput")
    deg_in = nc.dram_tensor("deg", [128, WINDOWS], mybir.dt.int32, kind="ExternalInput")
    dstoff_in = nc.dram_tensor("dstoff", [128, C_total], f32, kind="ExternalInput")
    gidx_in = nc.dram_tensor("gidx", [128, C_total * 8], mybir.dt.int16, kind="ExternalInput")
    wlT_in = nc.dram_tensor("W_lowT", [D, D], f32, kind="ExternalInput")
    whT_in = nc.dram_tensor("W_highT", [D, D], f32, kind="ExternalInput")
    wmT_in = nc.dram_tensor("W_midT", [D, D], f32, kind="ExternalInput")
    gl_in = nc.dram_tensor("gamma_low", [1, KG], f32, kind="ExternalInput")
    gh_in = nc.dram_tensor("gamma_high", [1, KG], f32, kind="ExternalInput")
    gm_in = nc.dram_tensor("gamma_mid", [1, KG], f32, kind="ExternalInput")
    bias_in = nc.dram_tensor("bias", [128, 1], f32, kind="ExternalInput")
    out_dram = nc.dram_tensor("out", [NPAD, D], f32, kind="ExternalOutput")

    alpha = np.linspace(EPS, 1.0 - EPS, KG)
    midalpha = np.linspace(EPS, 1.0, KG)

    with tile.TileContext(nc) as tc:
        with (
            tc.tile_pool(name="const", bufs=1) as constp,
            tc.tile_pool(name="big", bufs=1) as bigp,
            tc.tile_pool(name="msg", bufs=16) as msgp,
            tc.tile_pool(name="oh", bufs=8) as ohp,
            tc.tile_pool(name="wrk", bufs=4) as wrkp,
            tc.tile_pool(name="pswin", bufs=3, space="PSUM") as pswin,
            tc.tile_pool(name="psep", bufs=2, space="PSUM") as psep,
            tc.tile_pool(name="dram", bufs=1, space="DRAM") as dramp,
        ):
            # ---------------- constants / small tiles ----------------
            iota_i = constp.tile([128, 128], mybir.dt.int32)
            nc.gpsimd.iota(iota_i[:], pattern=[[1, 128]], base=0, channel_multiplier=0)
            iota_f = constp.tile([128, 128], f32)
            nc.vector.tensor_copy(iota_f[:], iota_i[:])
            ident = constp.tile([128, 128], f32)
            make_identity(nc, ident[:])

            dstoff_t = constp.tile([128, C_total], f32)
            nc.sync.dma_start(dstoff_t[:], dstoff_in[:])
            gidx_t = constp.tile([128, C_total * 8], mybir.dt.int16)
            nc.sync.dma_start(gidx_t[:], gidx_in[:])
            snorm_t = constp.tile([128, WINDOWS], f32)
            nc.sync.dma_start(snorm_t[:], snorm_in[:])
            bias_t = constp.tile([128, 1], f32)
            nc.sync.dma_start(bias_t[:], bias_in[:])
            wT = {}
            for nm, drt in (("low", wlT_in), ("high", whT_in), ("mid", wmT_in)):
                t = constp.tile([128, 128], f32, tag=f"w{nm}")
                nc.sync.dma_start(t[:], drt[:])
                wT[nm] = t

            # deg -> norm, norm^2   [128, WINDOWS]
            deg_t = constp.tile([128, WINDOWS], mybir.dt.int32)
            nc.sync.dma_start(deg_t[:], deg_in[:])
            deg_f = constp.tile([128, WINDOWS], f32)
            nc.vector.tensor_copy(deg_f[:], deg_t[:])
            nc.vector.tensor_scalar_max(deg_f[:], deg_f[:], 1.0)
            sq = constp.tile([128, WINDOWS], f32)
            nc.scalar.activation(sq[:], deg_f[:], AF.Sqrt)
            norm_t = constp.tile([128, WINDOWS], f32)
            nc.vector.reciprocal(norm_t[:], sq[:])
            norm2_t = constp.tile([128, WINDOWS], f32)
            nc.vector.tensor_tensor(norm2_t[:], norm_t[:], norm_t[:], OP.mult)

            # ---------------- epilogue scalar coefficients ----------------
            # broadcast gammas to [128, KG] via K=1 matmul with ones lhsT
            ones_row = constp.tile([1, 128], f32)
            nc.vector.memset(ones_row[:], 1.0)
            coeff = {}
            for nm, drt in (("low", gl_in), ("high", gh_in), ("mid", gm_in)):
                g_small = constp.tile([1, KG], f32, tag=f"gs{nm}")
                nc.sync.dma_start(g_small[:], drt[:])
                g_ps = psep.tile([128, KG], f32, space="PSUM", tag="uT")
                nc.tensor.matmul(g_ps[:], lhsT=ones_row[:], rhs=g_small[:],
                                 start=True, stop=True)
                g_b = constp.tile([128, KG], f32, tag=f"gb{nm}")
                nc.scalar.activation(g_b[:], g_ps[:], AF.Relu)
                coeff[nm] = g_b

            def dotcol(gb, weights, tag):
                wt = constp.tile([128, KG], f32, tag=f"wt{tag}")
                for i, v in enumerate(weights):
                    nc.vector.memset(wt[:, i:i + 1], float(v))
                prod = constp.tile([128, KG], f32, tag=f"pr{tag}")
                nc.vector.tensor_tensor(prod[:], gb[:], wt[:], OP.mult)
                col = constp.tile([128, 1], f32, tag=f"col{tag}")
                nc.vector.tensor_reduce(col[:], prod[:], mybir.AxisListType.X, OP.add)
                return col

            a0_col = dotcol(coeff["low"], alpha, "a0")
            b0_col = dotcol(coeff["low"], 1.0 - alpha, "b0")
            a1p_col = dotcol(coeff["high"], alpha, "a1p")   # positive; negated below
            b1_col = dotcol(coeff["high"], 1.0 - alpha, "b1")
            c2_col = dotcol(coeff["mid"], np.ones(KG), "c2")
            d2_col = dotcol(coeff["mid"], midalpha, "d2")
            a1_col = constp.tile([128, 1], f32)
            nc.vector.tensor_scalar_mul(a1_col[:], a1p_col[:], -1.0)

            # ---------------- load feature tiles ----------------
            x_buf = bigp.tile([128, NPAD], f32)   # window w at [:, w*128:(w+1)*128]
            nc.sync.dma_start(
                x_buf[:].rearrange("p (w d) -> p w d", d=D),
                feat_in[:].rearrange("(w p) d -> p w d", p=128),
            )

            # normfeat slice -> DRAM bounce, AllGather -> ag1
            bounce1 = dramp.tile([NPAD, D], bf16)
            for w in range(WINDOWS):
                nf = wrkp.tile([128, 128], bf16, tag="nf")
                nc.scalar.activation(nf[:], x_buf[:, w * 128:(w + 1) * 128], AF.Copy,
                                     scale=norm_t[:, w:w + 1])
                nc.sync.dma_start(bounce1[w * 128:(w + 1) * 128, :], nf[:])
            ag1 = dramp.tile([AGROWS, D], bf16)
            if skip_collectives:
                nc.gpsimd.dma_start(ag1[0:NPAD, :], bounce1[:, :])
            else:
                nc.gpsimd.collective_compute(
                    "AllGather", mybir.AluOpType.bypass,
                    ins=[bounce1.opt()], outs=[ag1.opt()],
                    replica_groups=[list(range(NCORES))],
                )

            h_buf = bigp.tile([128, NPAD], f32)
            h1_buf = bigp.tile([128, NPAD], f32)
            bounce2 = dramp.tile([NPAD, D], bf16)
            ag2 = dramp.tile([AGROWS, D], bf16)

            def mp_round(ag_src, out_h_buf, write_normh):
                """One message-passing round from gather table ag_src."""
                # issue all gather calls; msg tiles keyed by call index
                msg_tiles = {}
                dummy_msg = None
                if bench_mode == "scatter":
                    dummy_msg = msgp.tile([128, MAX_CALL_CHUNKS, 128], bf16, tag="msg")
                    nc.vector.memset(dummy_msg[:].rearrange("p c d -> p (c d)"), 0.5)
                for ci, (cs, nch, h) in enumerate(calls):
                    if bench_mode == "scatter":
                        for k in range(nch):
                            msg_tiles[cs + k] = (dummy_msg, k)
                        continue
                    mt = msgp.tile([128, MAX_CALL_CHUNKS, 128], bf16, tag="msg")
                    base = ag_src[SPLIT:, :] if h else ag_src[:SPLIT, :]
                    num_idxs = nch * 128
                    nc.gpsimd.dma_gather(
                        mt[:, :nch, :], base, gidx_t[:, cs * 8:cs * 8 + nch * 8],
                        num_idxs, num_idxs, D, queue_num=ci % NQUEUES,
                    )
                    if bench_mode == "gather":
                        red = ohp.tile([128, 1], f32, tag="gred")
                        nc.vector.tensor_reduce(
                            red[:], mt[:, 0, ::32], mybir.AxisListType.X,
                            OP.add)
                        nc.vector.tensor_tensor(
                            out_h_buf[:, 0:1], out_h_buf[:, 0:1], red[:], OP.add)
                        continue
                    for k in range(nch):
                        msg_tiles[cs + k] = (mt, k)
                if bench_mode == "gather":
                    return

                # chunk->window accumulation
                cur_w = -1
                psum_w = None
                wstart = {}
                for c0, (w, h) in enumerate(chunk_wh):
                    if w != cur_w:
                        cur_w = w
                        psum_w = pswin.tile([128, 128], f32, space="PSUM", tag="agg")
                        wstart[w] = True
                    mt, k = msg_tiles[c0]
                    oh = ohp.tile([128, 128], bf16, tag="oh")
                    nc.vector.tensor_scalar(oh[:], iota_f[:], dstoff_t[:, c0:c0 + 1],
                                            None, OP.is_equal)
                    last = (c0 + 1 == len(chunk_wh)) or chunk_wh[c0 + 1][0] != w
                    nc.tensor.matmul(psum_w[:], lhsT=oh[:], rhs=mt[:, k, :],
                                     start=wstart.pop(w, False), stop=last)
                    if last:
                        nc.scalar.activation(
                            out_h_buf[:, w * 128:(w + 1) * 128], psum_w[:],
                            AF.Copy, scale=norm_t[:, w:w + 1])
                        if write_normh:
                            nh = wrkp.tile([128, 128], bf16, tag="nh")
                            nc.scalar.activation(nh[:], psum_w[:], AF.Copy,
                                                 scale=norm2_t[:, w:w + 1])
                            nc.sync.dma_start(
                                bounce2[w * 128:(w + 1) * 128, :], nh[:])

            for _rep in range(repeat_body):
                if bench_mode == "epilogue":
                    nc.vector.memset(h_buf[:, 0:1], 0.1)
                    nc.vector.memset(h1_buf[:, 0:1], 0.1)
                elif bench_mode == "ag":
                    nc.gpsimd.dma_start(bounce2[0:128, :], x_buf[:, 0:128])
                    nc.gpsimd.collective_compute(
                        "AllGather", mybir.AluOpType.bypass,
                        ins=[bounce2.opt()], outs=[ag2.opt()],
                        replica_groups=[list(range(NCORES))],
                    )
                    continue
                else:
                    mp_round(ag1, h_buf, write_normh=True)
                    if skip_collectives:
                        nc.gpsimd.dma_start(ag2[0:NPAD, :], bounce2[:, :])
                    else:
                        nc.gpsimd.collective_compute(
                            "AllGather", mybir.AluOpType.bypass,
                            ins=[bounce2.opt()], outs=[ag2.opt()],
                            replica_groups=[list(range(NCORES))],
                        )
                    mp_round(ag2, h1_buf, write_normh=False)
                if bench_mode in ("gather", "scatter", "rounds"):
                    continue

                # ---------------- epilogue per window ----------------
                for w in range(WINDOWS):
                    sl = slice(w * 128, (w + 1) * 128)
                    x_w = x_buf[:, sl]
                    h_w = h_buf[:, sl]
                    h1_w = h1_buf[:, sl]

                    def combo(in_hi, a_col, x_col, op1, tag):
                        xb = wrkp.tile([128, 128], f32, tag=f"xb{tag}")
                        nc.vector.tensor_scalar_mul(xb[:], x_w, x_col[:])
                        u = wrkp.tile([128, 128], f32, tag=f"u{tag}")
                        nc.vector.scalar_tensor_tensor(
                            out=u[:], in0=in_hi, scalar=a_col[:], in1=xb[:],
                            op0=OP.mult, op1=op1)
                        return u

                    u0 = combo(h_w, a0_col, b0_col, OP.add, "0")
                    u1 = combo(h_w, a1_col, b1_col, OP.add, "1")
                    u2 = combo(h1_w, c2_col, d2_col, OP.subtract, "2")

                    oT = {}
                    for nm, u in (("low", u0), ("high", u1), ("mid", u2)):
                        up = psep.tile([128, 128], f32, space="PSUM", tag="uT")
                        nc.tensor.transpose(up[:], u[:], ident[:])
                        uT = wrkp.tile([128, 128], f32, tag=f"uT{nm}")
                        nc.vector.tensor_copy(uT[:], up[:])
                        op = psep.tile([128, 128], f32, space="PSUM", tag="om")
                        nc.tensor.matmul(op[:], lhsT=wT[nm][:], rhs=uT[:],
                                         start=True, stop=True)
                        ot = wrkp.tile([128, 128], f32, tag=f"ot{nm}")
                        nc.scalar.copy(ot[:], op[:])
                        oT[nm] = ot

                    # mutual gating (T layout)
                    tmp = wrkp.tile([128, 128], f32, tag="gt")
                    sig = wrkp.tile([128, 128], f32, tag="gs")
                    nc.vector.tensor_tensor(tmp[:], oT["high"][:], oT["mid"][:], OP.add)
                    nc.scalar.activation(sig[:], tmp[:], AF.Sigmoid)
                    nc.vector.tensor_tensor(oT["low"][:], oT["low"][:], sig[:], OP.mult)
                    nc.vector.tensor_tensor(tmp[:], oT["low"][:], oT["mid"][:], OP.add)
                    nc.scalar.activation(sig[:], tmp[:], AF.Sigmoid)
                    nc.vector.tensor_tensor(oT["high"][:], oT["high"][:], sig[:], OP.mult)
                    nc.vector.tensor_tensor(tmp[:], oT["low"][:], oT["high"][:], OP.add)
                    nc.scalar.activation(sig[:], tmp[:], AF.Sigmoid)
                    nc.vector.tensor_tensor(oT["mid"][:], oT["mid"][:], sig[:], OP.mult)

                    nc.vector.tensor_tensor(tmp[:], oT["low"][:], oT["high"][:], OP.add)
                    nc.vector.tensor_tensor(tmp[:], tmp[:], oT["mid"][:], OP.add)
                    nc.vector.tensor_scalar_add(tmp[:], tmp[:], bias_t[:])

                    # back to row layout; relu(x * snorm)
                    bp = psep.tile([128, 128], f32, space="PSUM", tag="uT")
                    nc.tensor.transpose(bp[:], tmp[:], ident[:])
                    outt = wrkp.tile([128, 128], f32, tag="outt")
                    nc.scalar.activation(outt[:], bp[:], AF.Relu,
                                         scale=snorm_t[:, w:w + 1])
                    nc.sync.dma_start(out_dram[w * 128:(w + 1) * 128, :], outt[:])

    nc.compile()
    return nc


# ---------------------------------------------------------------------------
# Public entry point
# ---------------------------------------------------------------------------

def kernel(feature, snorm_n, src, dst, W_low, W_high, W_mid,
           gamma_low, gamma_high, gamma_mid, bias):
    from concourse.bass_utils import run_bass_kernel_spmd

    feature = np.asarray(feature, dtype=np.float32)
    snorm_n = np.asarray(snorm_n, dtype=np.float32)
    schedule, per_core = _preprocess(np.asarray(src), np.asarray(dst))
    nc = _build(schedule)

    in_maps = []
    for c in range(NCORES):
        feat_c = np.zeros((NPAD, D), np.float32)
        feat_c[:NPC] = feature[c * NPC:(c + 1) * NPC]
        sn_c = np.zeros(NPAD, np.float32)
        sn_c[:NPC] = snorm_n[c * NPC:(c + 1) * NPC, 0]
        in_maps.append({
            "feature": feat_c,
            "snorm": sn_c.reshape(WINDOWS, 128).T.copy(),
            "deg": per_core[c]["deg"],
            "dstoff": per_core[c]["dstoff"],
            "gidx": per_core[c]["gidx"],
            "W_lowT": np.ascontiguousarray(np.asarray(W_low, np.float32).T),
            "W_highT": np.ascontiguousarray(np.asarray(W_high, np.float32).T),
            "W_midT": np.ascontiguousarray(np.asarray(W_mid, np.float32).T),
            "gamma_low": np.asarray(gamma_low, np.float32).reshape(1, KG),
            "gamma_high": np.asarray(gamma_high, np.float32).reshape(1, KG),
            "gamma_mid": np.asarray(gamma_mid, np.float32).reshape(1, KG),
            "bias": np.asarray(bias, np.float32).reshape(128, 1),
        })

    res = run_bass_kernel_spmd(nc, in_maps, core_ids=list(range(NCORES)))
    out = np.concatenate(
        [res.results[c]["out"][:NPC] for c in range(NCORES)], axis=0)
    return out
